# revision 2
# baseline (speedup 1.0000x reference)
"""GraphSAGE GNN Bass kernel for TRN2, 8-core SPMD — v2.

Design (dst-partitioned, SBUF-slice-resident gather):
  - Core c owns dsts [c*V, (c+1)*V). VP = V padded to 512-multiple; nodes
    split into 4 quarters (buckets) of Q=VP/4 local rows.
  - Tables: per (layer, bucket) a DRAM tensor tblag [NC, 128, QT, H] bf16:
    block-major stripe layout; core c's block [128, QT, H] holds its quarter
    with node (local row lr) at [lr%128, lr//128, :]. Built by 4 chunked
    AllGathers per layer (overlaps next layer's compute); layer 0 host-built.
  - Aggregation pass b (b=0..3): DMA the whole bucket table into SBUF
    (fat descriptors), then SBUF-source dma_gather (0.43 ns/desc vs 5.5
    HBM) expands edge slots: F^T [feat, slot] tiles; PE-transpose back to
    slot-major; one-hot M (fp8, exact) matmuls accumulate 64-dst cells
    into PSUM; psum tiles added into SBUF acc [128, NT, H] f32 across the
    4 bucket passes. Uniform SPMD schedule: cell block counts K[b][u] =
    max over cores (zero-padded M/idx for cores with fewer edges).
  - Transform: z = Wl^T(inv_deg*aggT) + Wr^T(hT) fp32; zT overwrites acc;
    BN stats AllReduce; affine(+ReLU) -> hT; stage quarters; 4 AllGathers.
  - Classifier per core on hT.
"""

import numpy as np
import ml_dtypes
import concourse.bass as bass
import concourse.tile as tile
from concourse import bacc, mybir
from concourse.masks import make_identity

F32 = mybir.dt.float32
BF16 = mybir.dt.bfloat16
F8 = mybir.dt.float8e4
I16 = mybir.dt.int16
F8NP = ml_dtypes.float8_e4m3
MDT = mybir.dt.bfloat16          # M matrix device dtype
MNP = ml_dtypes.bfloat16

NC = 8          # cores
NBK = 4         # src buckets (= quarters)
H = 128
BLK = 128       # slots per block
PIECE_B = 32    # blocks per gather piece (4096 idxs)
CELL = 64       # dsts per cell (matmul lhsT width)
EPS = 1e-5
NSWQ = 4        # SWDGE queues


class Cfg2:
    def __init__(self, N, E, d_in=12):
        self.N, self.E, self.d_in = N, E, d_in
        assert N % NC == 0
        self.V = N // NC
        self.VP = ((self.V + 511) // 512) * 512
        self.Q = self.VP // 4            # local rows per quarter
        self.QT = self.Q // 128          # ranks per quarter
        self.NT = self.VP // 128         # 128-dst tiles
        self.TL = self.VP // CELL        # 64-dst cells
        self.BROWS = NC * self.Q         # bucket table rows
        assert self.BROWS <= 32768       # int16 gather idx
        self.tf_tiles = [(i, min(512, self.V - i)) for i in range(0, self.V, 512)]


def preprocess2(edge_index, cfg: Cfg2):
    src = np.asarray(edge_index[0], np.int64)
    dst = np.asarray(edge_index[1], np.int64)
    N, V, VP, Q = cfg.N, cfg.V, cfg.VP, cfg.Q
    deg = np.bincount(dst, minlength=N).astype(np.float32)
    inv_deg = (np.float32(1.0) / np.maximum(deg, np.float32(1.0))).astype(np.float32)

    c_src = src // V
    l_src = src % V
    b_src = np.minimum(l_src // Q, NBK - 1)
    rb_src = c_src * Q + (l_src - b_src * Q)      # bucket row id
    c_dst = dst // V
    d_loc = dst % V

    # per (core, bucket): edge lists sorted by local dst
    cell_counts = np.zeros((NC, NBK, cfg.TL), np.int64)
    ed = {}
    for c in range(NC):
        mc = c_dst == c
        for b in range(NBK):
            m = mc & (b_src == b)
            d = d_loc[m]
            r = rb_src[m]
            o = np.argsort(d, kind="stable")
            d, r = d[o], r[o]
            ed[(c, b)] = (d, r)
            cell_counts[c, b] = np.bincount(d // CELL, minlength=cfg.TL)

    # uniform block schedule: K[b][u] = max over cores, >= 1
    K = np.maximum(np.ceil(cell_counts.max(axis=0) / BLK).astype(np.int64), 1)
    B_pass = K.sum(axis=1)                        # blocks per bucket pass
    P_pass = [int(-(-bp // PIECE_B)) for bp in B_pass]
    P_tot = int(sum(P_pass))
    blk_tot = P_tot * PIECE_B

    pre = dict(K=K, P_pass=P_pass, P_tot=P_tot, inv_deg=inv_deg,
               gidx=[], mmat=[], invd=[])
    prefix = np.zeros((NBK, cfg.TL), np.int64)    # block offset of cell start
    for b in range(NBK):
        prefix[b] = np.concatenate([[0], np.cumsum(K[b])[:-1]])

    for c in range(NC):
        gidx = np.zeros(blk_tot * BLK, np.int16)
        mm = np.zeros((blk_tot, BLK, CELL), MNP)
        blk0 = 0
        for b in range(NBK):
            d, r = ed[(c, b)]
            cells = d // CELL
            cstart = np.concatenate([[0], np.cumsum(np.bincount(
                cells, minlength=cfg.TL))[:-1]])
            j = np.arange(d.size) - cstart[cells]         # rank within cell
            slot = (blk0 + prefix[b][cells]) * BLK + j
            gidx[slot] = r.astype(np.int16)
            gblk = blk0 + prefix[b][cells] + j // BLK
            mm[gblk, j % BLK, d - cells * CELL] = inv_deg[c * V + d]
            blk0 += P_pass[b] * PIECE_B
        pre["gidx"].append(_wrap16(gidx).reshape(128, P_tot, 256)
                           .transpose(1, 0, 2).copy())
        pre["mmat"].append(mm.reshape(P_tot, PIECE_B, BLK, CELL)
                           .transpose(0, 2, 1, 3)
                           .reshape(P_tot, 128, PIECE_B * CELL).copy())
        it = np.ones((128, cfg.NT), np.float32)
        vr = np.arange(VP)
        vv = vr < V
        it[vr[vv] % 128, vr[vv] // 128] = inv_deg[c * V + vr[vv]]
        pre["invd"].append(np.ascontiguousarray(it))
    return pre


def _wrap16(flat):
    """[L] -> [128, L/16]: element i at [i%16, i//16], replicated 8x."""
    assert flat.size % 16 == 0
    return np.tile(np.ascontiguousarray(flat.reshape(-1, 16).T), (8, 1))


def _stripe_table(vals, cfg):
    """vals [N, H] -> per-bucket row-major tables [NBK][NC*Q, H]."""
    V, Q = cfg.V, cfg.Q
    out = []
    for b in range(NBK):
        t = np.zeros((NC * Q, H), vals.dtype)
        for c in range(NC):
            lr0 = b * Q
            n = min(V - lr0, Q) if V > lr0 else 0
            if n <= 0:
                continue
            t[c * Q: c * Q + n] = vals[c * V + lr0: c * V + lr0 + n]
        out.append(t)
    return out


def build_inputs2(inputs, pre, cfg: Cfg2):
    V, VP, d_in = cfg.V, cfg.VP, cfg.d_in
    x = np.asarray(inputs["x"], np.float32)
    xpad = np.zeros((cfg.N, H), np.float32)
    xpad[:, :d_in] = x
    tbl0 = _stripe_table(xpad.astype(ml_dtypes.bfloat16), cfg)

    def padT(w, rows, cols):
        o = np.zeros((rows, cols), np.float32)
        o[: w.shape[0], : w.shape[1]] = w
        return o

    Wl0 = np.asarray(inputs["Wl0"], np.float32)
    Wr0 = np.asarray(inputs["Wr0"], np.float32)
    Wl = np.asarray(inputs["Wl"], np.float32)
    Wr = np.asarray(inputs["Wr"], np.float32)
    wlT = np.stack([padT(Wl0.T, H, H), Wl[0].T, Wl[1].T]).astype(np.float32)
    wrT = np.stack([padT(Wr0.T, H, H), Wr[0].T, Wr[1].T]).astype(np.float32)
    gam = np.ascontiguousarray(np.asarray(inputs["gamma"], np.float32).T)
    bet = np.ascontiguousarray(np.asarray(inputs["beta"], np.float32).T)
    wc1T = np.ascontiguousarray(np.asarray(inputs["Wc1"], np.float32).T)
    bc1 = np.asarray(inputs["bc1"], np.float32).reshape(-1, 1)
    wc2T = np.ascontiguousarray(np.asarray(inputs["Wc2"], np.float32).T)
    bc2 = np.asarray(inputs["bc2"], np.float32).reshape(1, 1)

    in_maps = []
    for c in range(NC):
        xT = np.zeros((H, VP), np.float32)
        xT[:d_in, :V] = x[c * V:(c + 1) * V].T
        m = dict(xT=xT, gidx=pre["gidx"][c], mmat=pre["mmat"][c],
                 invd=pre["invd"][c], wlT=wlT, wrT=wrT, gam=gam, bet=bet,
                 wc1T=wc1T, bc1=bc1, wc2T=wc2T, bc2=bc2)
        for b in range(NBK):
            m[f"tbl0q{b}"] = tbl0[b]
        in_maps.append(m)
    return in_maps


def build_program2(cfg: Cfg2, pre, layers=3, reps=1, stop=None):
    V, VP, Q, QT, NT, TL = cfg.V, cfg.VP, cfg.Q, cfg.QT, cfg.NT, cfg.TL
    K, P_pass, P_tot = pre["K"], pre["P_pass"], pre["P_tot"]

    nc = bacc.Bacc("TRN2", target_bir_lowering=False, debug=False,
                   num_devices=NC, num_swdge_queues=NSWQ)

    ext = {}
    def ein(name, shape, dt):
        ext[name] = nc.dram_tensor(name, shape, dt, kind="ExternalInput")
        return ext[name]

    tbl0q = [ein(f"tbl0q{b}", [NC * Q, H], BF16) for b in range(NBK)]
    xT_e = ein("xT", [H, VP], F32)
    gidx_e = ein("gidx", [P_tot, 128, 256], I16)
    mmat_e = ein("mmat", [P_tot, 128, PIECE_B * CELL], MDT)
    invd_e = ein("invd", [128, NT], F32)
    wlT_e = ein("wlT", [3, H, H], F32)
    wrT_e = ein("wrT", [3, H, H], F32)
    gam_e = ein("gam", [H, 3], F32)
    bet_e = ein("bet", [H, 3], F32)
    wc1T_e = ein("wc1T", [H, 64], F32)
    bc1_e = ein("bc1", [64, 1], F32)
    wc2T_e = ein("wc2T", [64, 1], F32)
    bc2_e = ein("bc2", [1, 1], F32)
    logits_e = nc.dram_tensor("logits", [1, VP], F32, kind="ExternalOutput")
    dbg_e = (nc.dram_tensor("dbg", [128, NT * H], F32, kind="ExternalOutput")
             if stop == "AGG" else None)

    # internal DRAM: staged quarters + gathered tables for layers 1,2
    agin = [[nc.dram_tensor(f"agin{l}q{b}", [Q, H], BF16)
             for b in range(NBK)] for l in range(2)]
    tblag = [[nc.dram_tensor(f"tblag{l}q{b}", [NC * Q, H], BF16,
                             addr_space="Shared")
              for b in range(NBK)] for l in range(2)]
    arin = [nc.dram_tensor(f"arin{l}", [H, 2], F32) for l in range(3)]
    arout = [nc.dram_tensor(f"arout{l}", [H, 2], F32, addr_space="Shared")
             for l in range(3)]
    rg = [list(range(NC))]

    with tile.TileContext(nc) as tc:
        import contextlib
        cm = contextlib.ExitStack()
        with cm:
            singles = cm.enter_context(tc.tile_pool(name="singles", bufs=1))
            persist = cm.enter_context(tc.tile_pool(name="persist", bufs=1))
            ftp = cm.enter_context(tc.tile_pool(name="ftp", bufs=6))
            mp = cm.enter_context(tc.tile_pool(name="mp", bufs=4))
            gp = cm.enter_context(tc.tile_pool(name="gp", bufs=3))
            scp = cm.enter_context(tc.tile_pool(name="scp", bufs=3))
            aggp = cm.enter_context(tc.tile_pool(name="aggp", bufs=2))
            scr = cm.enter_context(tc.tile_pool(name="scr", bufs=2))
            small = cm.enter_context(tc.tile_pool(name="small", bufs=2))
            stgp = cm.enter_context(tc.tile_pool(name="stgp", bufs=3))
            ps_agg = cm.enter_context(tc.tile_pool(name="ps_agg", bufs=2, space="PSUM"))
            ps_big = cm.enter_context(tc.tile_pool(name="ps_big", bufs=2, space="PSUM"))

            # ---- constants ----
            wlT = singles.tile([H, 3, H], F32, tag="wlT")
            wrT = singles.tile([H, 3, H], F32, tag="wrT")
            nc.sync.dma_start(out=wlT[:], in_=wlT_e[:].rearrange("l k m -> k l m"))
            nc.sync.dma_start(out=wrT[:], in_=wrT_e[:].rearrange("l k m -> k l m"))
            gam = singles.tile([H, 3], F32, tag="gam")
            bet = singles.tile([H, 3], F32, tag="bet")
            nc.sync.dma_start(out=gam[:], in_=gam_e[:])
            nc.sync.dma_start(out=bet[:], in_=bet_e[:])
            wc1T = singles.tile([H, 64], F32, tag="wc1T")
            nc.sync.dma_start(out=wc1T[:], in_=wc1T_e[:])
            bc1 = singles.tile([64, 1], F32, tag="bc1")
            nc.sync.dma_start(out=bc1[:], in_=bc1_e[:])
            wc2T = singles.tile([64, 1], F32, tag="wc2T")
            nc.sync.dma_start(out=wc2T[:], in_=wc2T_e[:])
            bc2 = singles.tile([1, 1], F32, tag="bc2")
            nc.sync.dma_start(out=bc2[:], in_=bc2_e[:])
            invd = singles.tile([128, NT], F32, tag="invd")
            nc.sync.dma_start(out=invd[:], in_=invd_e[:])
            ident = singles.tile([128, 128], F32, tag="ident")
            make_identity(nc, ident[:])
            epsT = singles.tile([128, 1], F32, tag="epsT")
            nc.vector.memset(epsT[:], EPS)

            # ---- persistent buffers ----
            hT = persist.tile([H, VP], F32, tag="hT")
            acc = persist.tile([128, VP], F32, tag="acc")   # aggT, then zT

            for rep in range(reps):
                nc.sync.dma_start(out=hT[:], in_=xT_e[:])

                for layer in range(layers):
                    # ===== aggregation: 4 bucket passes =====
                    n_tf = len(cfg.tf_tiles)
                    sums = small.tile([128, n_tf], F32, tag="sums")
                    sumsq = small.tile([128, n_tf], F32, tag="sumsq")

                    def emit_transform(ti):
                        c0, nt = cfg.tf_tiles[ti]
                        pz = ps_big.tile([128, 512], F32, tag="pz")
                        nc.tensor.matmul(pz[:, :nt], wlT[:, layer, :],
                                         acc[:, c0:c0 + nt], start=True, stop=False)
                        nc.tensor.matmul(pz[:, :nt], wrT[:, layer, :],
                                         hT[:, c0:c0 + nt], start=False, stop=True)
                        zt = scr.tile([128, 512], F32, tag="zt")
                        nc.vector.tensor_copy(out=zt[:, :nt], in_=pz[:, :nt])
                        # stash zT over acc (acc region for this chunk is dead)
                        nc.vector.tensor_copy(out=acc[:, c0:c0 + nt],
                                              in_=zt[:, :nt])
                        nc.vector.reduce_sum(out=sums[:, ti:ti + 1], in_=zt[:, :nt],
                                             axis=mybir.AxisListType.X)
                        sq = scr.tile([128, 512], F32, tag="sq")
                        nc.vector.tensor_mul(sq[:, :nt], zt[:, :nt], zt[:, :nt])
                        nc.vector.reduce_sum(out=sumsq[:, ti:ti + 1], in_=sq[:, :nt],
                                             axis=mybir.AxisListType.X)

                    blk_base = 0
                    for b in range(NBK):
                        src_tbl = tbl0q[b] if layer == 0 else tblag[layer - 1][b]

                        sched = [(u, k) for u in range(TL) for k in range(int(K[b][u]))]
                        B_real = len(sched)
                        bi = 0              # block index within pass
                        cur_pair = -1       # 512-dst psum group index
                        psum_cur = None

                        def flush_pair():
                            nonlocal psum_cur, cur_pair
                            if psum_cur is None:
                                return
                            g0 = cur_pair * 512
                            gw = min(512, VP - g0)
                            if b == 0:
                                nc.vector.tensor_copy(out=acc[:, g0:g0 + gw],
                                                      in_=psum_cur[:, :gw])
                            else:
                                nc.vector.tensor_add(acc[:, g0:g0 + gw],
                                                     acc[:, g0:g0 + gw],
                                                     psum_cur[:, :gw])
                            if (b == NBK - 1 and stop not in ("G", "A")
                                    and cur_pair < len(cfg.tf_tiles)):
                                emit_transform(cur_pair)
                            psum_cur = None
                            cur_pair = -1

                        for p in range(P_pass[b]):
                            pg = blk_base + p
                            ft = ftp.tile([128, PIECE_B, H], BF16, tag="ft")
                            gpt = gp.tile([128, 256], I16, tag="gpt")
                            nc.sync.dma_start(out=gpt[:], in_=gidx_e[pg])
                            nc.gpsimd.dma_gather(
                                out_ap=ft[:], in_ap=src_tbl[:, :],
                                idxs_ap=gpt[:],
                                num_idxs=PIECE_B * BLK,
                                num_idxs_reg=PIECE_B * BLK,
                                elem_size=H,
                                single_packet=False, queue_num=pg % NSWQ,
                            )
                            if stop == "G":
                                continue
                            mt = mp.tile([128, PIECE_B * CELL], MDT, tag="mt")
                            nc.sync.dma_start(out=mt[:], in_=mmat_e[pg])
                            # matmuls for this piece's blocks
                            for kk in range(PIECE_B):
                                if bi >= B_real:
                                    break
                                u, k = sched[bi]
                                g = u // 8
                                if g != cur_pair:
                                    flush_pair()
                                    psum_cur = ps_agg.tile([128, 512], F32, tag="pag")
                                    cur_pair = g
                                off = (u % 8) * CELL
                                nc.tensor.matmul(
                                    psum_cur[:, off:off + CELL],
                                    ft[:, kk, :],
                                    mt[:, kk * CELL:(kk + 1) * CELL],
                                    start=(k == 0), stop=(k == int(K[b][u]) - 1),
                                )
                                bi += 1
                        if stop != "G":
                            flush_pair()
                        blk_base += P_pass[b]

                    if stop == "AGG" and layer == layers - 1:
                        nc.sync.dma_start(out=dbg_e[:], in_=acc[:])
                        lz = small.tile([1, 512], F32, tag="lsb")
                        nc.vector.memset(lz[:], 0.0)
                        nc.sync.dma_start(out=logits_e[:, :512], in_=lz[:])
                        break
                    if stop in ("G", "A") and layer == layers - 1:
                        lz = small.tile([1, 512], F32, tag="lsb")
                        nc.vector.memset(lz[:], 0.0)
                        nc.sync.dma_start(out=logits_e[:, :512], in_=lz[:])
                        break

                    # ===== transform interleaved into pass 3; stats here =====
                    stats2 = small.tile([128, 2], F32, tag="stats2")
                    nc.vector.reduce_sum(out=stats2[:, 0:1], in_=sums[:],
                                         axis=mybir.AxisListType.X)
                    nc.vector.reduce_sum(out=stats2[:, 1:2], in_=sumsq[:],
                                         axis=mybir.AxisListType.X)
                    if stop == "T" and layer == layers - 1:
                        lz = small.tile([1, 512], F32, tag="lsb")
                        nc.vector.memset(lz[:], 0.0)
                        nc.sync.dma_start(out=logits_e[:, :512], in_=lz[:])
                        break
                    nc.sync.dma_start(out=arin[layer][:], in_=stats2[:])
                    nc.gpsimd.collective_compute(
                        "AllReduce", mybir.AluOpType.add, replica_groups=rg,
                        ins=[arin[layer][:]], outs=[arout[layer][:]])
                    gstat = small.tile([128, 2], F32, tag="gstat")
                    nc.sync.dma_start(out=gstat[:], in_=arout[layer][:])
                    mean = small.tile([128, 1], F32, tag="mean")
                    va = small.tile([128, 1], F32, tag="va")
                    aa = small.tile([128, 1], F32, tag="aa")
                    cc = small.tile([128, 1], F32, tag="cc")
                    nc.vector.tensor_scalar_mul(mean[:], gstat[:, 0:1], 1.0 / cfg.N)
                    nc.vector.tensor_scalar_mul(va[:], gstat[:, 1:2], 1.0 / cfg.N)
                    nc.vector.tensor_mul(cc[:], mean[:], mean[:])
                    nc.vector.tensor_sub(va[:], va[:], cc[:])
                    nc.scalar.activation(out=va[:], in_=va[:],
                                         func=mybir.ActivationFunctionType.Sqrt,
                                         bias=epsT[:], scale=1.0)
                    nc.vector.reciprocal(va[:], va[:])
                    nc.vector.tensor_mul(aa[:], gam[:, layer:layer + 1], va[:])
                    nc.vector.tensor_mul(cc[:], mean[:], aa[:])
                    nc.vector.tensor_sub(cc[:], bet[:, layer:layer + 1], cc[:])

                    # ===== affine (+relu) from zT (in acc) into hT =====
                    for (c0, nt) in cfg.tf_tiles:
                        if layer < 2:
                            nc.scalar.activation(out=hT[:, c0:c0 + nt],
                                                 in_=acc[:, c0:c0 + nt],
                                                 func=mybir.ActivationFunctionType.Relu,
                                                 bias=cc[:], scale=aa[:])
                        else:
                            nc.vector.tensor_scalar(out=hT[:, c0:c0 + nt],
                                                    in0=acc[:, c0:c0 + nt],
                                                    scalar1=aa[:], scalar2=cc[:],
                                                    op0=mybir.AluOpType.mult,
                                                    op1=mybir.AluOpType.add)

                    if stop == "AF" and layer == layers - 1:
                        lz = small.tile([1, 512], F32, tag="lsb")
                        nc.vector.memset(lz[:], 0.0)
                        nc.sync.dma_start(out=logits_e[:, :512], in_=lz[:])
                        break
                    # ===== stage + chunked AllGather =====
                    if layer < 2:
                        for b in range(NBK):
                            for r0 in range(0, QT, 4):
                                nr = min(4, QT - r0)
                                pst = ps_big.tile([128, 512], F32, tag="pz")
                                for q in range(nr):
                                    t = b * QT + r0 + q
                                    nc.tensor.transpose(
                                        out=pst[:, q * 128:(q + 1) * 128],
                                        in_=hT[:, t * 128:(t + 1) * 128],
                                        identity=ident[:])
                                stg = stgp.tile([128, 4, H], BF16, tag="stg")
                                nc.vector.tensor_copy(
                                    out=stg[:, :nr, :],
                                    in_=pst[:, :nr * 128].rearrange(
                                        "p (q f) -> p q f", q=nr))
                                nc.sync.dma_start(
                                    out=agin[layer][b][r0 * 128:(r0 + nr) * 128, :]
                                    .rearrange("(q p) f -> p q f", p=128),
                                    in_=stg[:, :nr, :])
                            nc.gpsimd.collective_compute(
                                "AllGather", mybir.AluOpType.bypass,
                                replica_groups=rg,
                                ins=[agin[layer][b][:]], outs=[tblag[layer][b][:]])

            # ===== classifier =====
            for (c0, nt) in cfg.tf_tiles:
                pc1 = ps_big.tile([128, 512], F32, tag="pz")
                nc.tensor.matmul(pc1[:64, :nt], wc1T[:], hT[:, c0:c0 + nt],
                                 start=True, stop=True)
                h3 = scr.tile([128, 512], F32, tag="sq")
                nc.scalar.activation(out=h3[:64, :nt], in_=pc1[:64, :nt],
                                     func=mybir.ActivationFunctionType.Relu,
                                     bias=bc1[:], scale=1.0)
                pc2 = ps_big.tile([128, 512], F32, tag="pz")
                nc.tensor.matmul(pc2[:1, :nt], wc2T[:], h3[:64, :nt],
                                 start=True, stop=True)
                lsb = small.tile([1, 512], F32, tag="lsb")
                nc.vector.tensor_scalar_add(lsb[:, :nt], pc2[:1, :nt], bc2[:])
                nc.sync.dma_start(out=logits_e[:, c0:c0 + nt], in_=lsb[:, :nt])

    nc.compile()
    return nc


# ======================= harness entry point =======================

def _run_with_retry(nc, in_maps, cores, tries=3):
    from concourse.bass_utils import run_bass_kernel_spmd
    last = None
    for _ in range(tries):
        try:
            return run_bass_kernel_spmd(nc, in_maps, cores)
        except Exception as e:  # transient axon terminal failures
            last = e
    raise last


def kernel(**inputs):
    """Full-input entry: shards across 8 NeuronCores internally."""
    cfg = Cfg2(N=100000, E=3200000)
    edge_index = np.asarray(inputs["edge_index"])
    pre = preprocess2(edge_index, cfg)
    in_maps = build_inputs2(inputs, pre, cfg)
    nc = build_program2(cfg, pre)
    res = _run_with_retry(nc, in_maps, list(range(NC)))
    logits = np.concatenate(
        [np.asarray(res.results[c]["logits"])[0, :cfg.V] for c in range(NC)]
    ).astype(np.float32)
    return logits


def _time_nc(nc, in_maps, n_cores=NC, reps=6):
    import time
    import jax
    from jax.sharding import Mesh, PartitionSpec, NamedSharding
    from jax.experimental.shard_map import shard_map
    from concourse import bass2jax

    bass2jax.install_neuronx_cc_hook()
    in_names, out_names, out_avals, zero_outs = [], [], [], []
    for alloc in nc.m.functions[0].allocations:
        if not isinstance(alloc, mybir.MemoryLocationSet):
            continue
        name = alloc.memorylocations[0].name
        if alloc.kind == "ExternalInput":
            if nc.partition_id_tensor is not None and name == nc.partition_id_tensor.name:
                continue
            in_names.append(name)
        elif alloc.kind == "ExternalOutput":
            shape = tuple(alloc.tensor_shape)
            dtype = mybir.dt.np(alloc.dtype)
            out_names.append(name)
            out_avals.append(jax.core.ShapedArray(shape, dtype))
            zero_outs.append(np.zeros(shape, dtype))
    n_params = len(in_names)
    all_in_names = in_names + out_names
    if nc.partition_id_tensor is not None:
        all_in_names.append(nc.partition_id_tensor.name)
    donate = tuple(range(n_params, n_params + len(out_names)))

    def _body(*args):
        ops = list(args)
        if nc.partition_id_tensor is not None:
            ops.append(bass2jax.partition_id_tensor())
        return tuple(bass2jax._bass_exec_p.bind(
            *ops, out_avals=tuple(out_avals), in_names=tuple(all_in_names),
            out_names=tuple(out_names), lowering_input_output_aliases=(),
            sim_require_finite=True, sim_require_nnan=True, nc=nc))

    mesh = Mesh(np.asarray(jax.devices()[:n_cores]), ("core",))
    sharded = jax.jit(shard_map(_body, mesh=mesh,
                                in_specs=(PartitionSpec("core"),) * (n_params + len(out_names)),
                                out_specs=(PartitionSpec("core"),) * len(out_names),
                                check_rep=False),
                      donate_argnums=donate, keep_unused=True)
    sh = NamedSharding(mesh, PartitionSpec("core"))
    dev_in = [jax.device_put(np.concatenate(
        [np.asarray(in_maps[c][nm]) for c in range(n_cores)], axis=0), sh)
        for nm in in_names]
    for d in dev_in:
        d.block_until_ready()
    walls = []
    outs = None
    for _ in range(reps + 1):
        zeros = [jax.device_put(np.zeros((n_cores * z.shape[0], *z.shape[1:]), z.dtype), sh)
                 for z in zero_outs]
        for z in zeros:
            z.block_until_ready()
        t0 = time.time()
        outs = sharded(*dev_in, *zeros)
        for o in outs:
            o.block_until_ready()
        walls.append(time.time() - t0)
    return walls, outs, out_names


def benchmark(inputs, reps=6):
    """Device time via K-repetition slope: build the forward repeated K times
    inside one program; (wall(K_hi)-wall(K_lo))/(K_hi-K_lo) cancels the axon
    per-call dispatch floor. Returns (est_device_ns, logits)."""
    cfg = Cfg2(N=100000, E=3200000)
    pre = preprocess2(np.asarray(inputs["edge_index"]), cfg)
    in_maps = build_inputs2(inputs, pre, cfg)
    best = {}
    outs = out_names = None
    for k in (1, 3):
        nck = build_program2(cfg, pre, layers=3, reps=k)
        walls, o, onames = _time_nc(nck, in_maps, reps=reps)
        best[k] = min(walls[1:])
        print(f"K={k} per-call walls (s): {[round(w, 4) for w in walls]}", flush=True)
        if k == 1:
            outs, out_names = o, onames
    est_ns = max((best[3] - best[1]) / 2.0, 1e-6) * 1e9
    la = np.asarray(outs[out_names.index("logits")]).reshape(NC, 1, cfg.VP)
    logits = np.concatenate([la[c, 0, :cfg.V] for c in range(NC)]).astype(np.float32)
    return est_ns, logits
